# revision 64
# baseline (speedup 1.0000x reference)
"""Trainium2 Bass kernel for ContinuousREWAEncoder:
    out = FWHT(x @ W^T)/sqrt(32) + 0.01*normal(key=42)

Math folding: FWHT is linear => out = x @ (H @ W / sqrt(32))^T + noise.
The noise uses a fixed PRNG key, so it is a deterministic constant computed
on host (with the same jax op/backend as the reference) and added in the
host epilogue (with the layout unpermute), keeping it off the HBM stream.

Sharding: pure data parallel over tokens (B*N = 32768 -> 4096/core on 8
cores). W_eff is replicated.

The kernel is HBM-bound, so x streams as fp8e3 (e3m4: 4 mantissa bits) —
half the bytes of fp16 — while W stays fp16 (mixed-dtype matmul). Measured
absmax rel err vs the fp32 reference ~1.1e-2 (gate 2e-2). Output moves as
fp16.

Device schedule per core (TOK=4096, supersteps of 1536/1536/1024 tokens,
each = 4 PE column groups):
  - all x rides the sync HWDGE ring as 7 DMAs in consumption order
    (768 KB halves of the full supersteps; the last superstep as
    chunks 0-5 [768 KB], chunk 6 [128 KB], chunk 7 [128 KB]) so only a
    small piece arrives last. Pieces stay >=1 KB per partition —
    smaller-run descriptors measured intermittent multi-us drain crawls.
  - w and the early out store ride the scalar HWDGE ring; the final
    64 KB store rides sync (idle once the last x piece drains, while
    scalar may still hold the early store's descriptors). Splitting the
    x stream itself across both rings measured WORSE: SDMA service
    between queues is unfair and the whole-DMA completion semaphore
    turned a trickling scalar-side piece into a 4us PE stall.
  - per superstep: 8 k-chunks x 4 col groups of matmuls (N = tokens/4),
    the 4 groups running concurrently in the PE column groups
    (tile_position), accumulating into that superstep's own PSUM bank;
    then a [128, N] DVE cast. Separate banks let each superstep's
    cast pipeline under the continuing x stream (PE-write/DVE-read on
    one bank serialize, so sub-bank splits can't).
  - three supersteps keep the PE program at ~200 instructions — under
    one 16 KiB IRAM block. Four or more supersteps overflowed it and a
    mid-kernel instruction fetch stalled the PE 1-3us at the tail
    (run-to-run variance of +-2us; now +-0.3us).
  - supersteps 0-1 cast into one shared SBUF tile stored as a single
    192 KB DMA; the last superstep stores separately (64 KB) so after
    the final x byte only 4 concurrent N=256 matmuls, one [128,256]
    cast and the 64 KB store remain (~2.8us tail).
  - measured anatomy per run (~25.5us total): ~6.6us fixed runtime/NEFF
    entry (engine init + program load + barriers), ~1.45us issue-to-
    first-byte, ~11.5us x stream at 360-430 GB/s (HBM-bound), ~2.8us
    tail chain, ~2.8us fixed exit (drain + semaphore teardown).
"""

import math
import sys
import types

import numpy as np
import ml_dtypes

import concourse.tile as tile
from concourse import bacc, mybir
from concourse.bass_utils import run_bass_kernel_spmd


def _ensure_axon_hooks():
    """run_bass_kernel_spmd's trace path (taken whenever env BASS_TRACE is
    set, even without an explicit trace=True) imports antenv.axon_hooks,
    which this image's antenv stub lacks. Provide it so the kernel works
    regardless of the caller's environment; register the real NTFF hook if
    the boot helper is available (hook=None degrades to a no-trace run)."""
    try:
        import antenv

        if not hasattr(antenv, "axon_hooks"):
            mod = types.ModuleType("antenv.axon_hooks")
            mod._hook = None
            mod.set_axon_ntff_profile_hook = lambda h: setattr(mod, "_hook", h)
            mod.get_axon_ntff_profile_hook = lambda: mod._hook
            sys.modules["antenv.axon_hooks"] = mod
            antenv.axon_hooks = mod
            # hook stays None: run_bass_kernel_spmd then degrades to an
            # untraced run instead of crashing on the missing module. A
            # harness that wants NTFF traces registers its own hook here.
    except Exception:
        pass


_ensure_axon_hooks()

B, N, D, M = 4, 8192, 1024, 32
NOISE_STD = 0.01
N_CORES = 8
TOK_TOTAL = B * N              # 32768
TOK = TOK_TOTAL // N_CORES     # 4096 tokens per core
NGRP = 4                       # col groups per superstep (PE col tiling)
KC = D // 128                  # 8 contraction chunks
# Three supersteps keep the PE program under 256 instructions (one 16 KiB
# IRAM block): 96 matmuls + 96 ldweights. Four supersteps measured an
# intermittent mid-kernel IRAM refill (16 KB runtime DMA late in the
# stream) that stalls the PE 1-3us right at the tail.
SS_TOK = [1536, 1536, 1024]             # tokens per superstep
SS_BLK = [t // NGRP for t in SS_TOK]    # 384, 384, 256
SS = len(SS_TOK)
assert sum(SS_TOK) == TOK

X_DT = mybir.dt.float8e3       # e3m4: 1 byte, 4 mantissa bits
X_NP = ml_dtypes.float8_e3m4
W_DT = mybir.dt.float16
F16 = mybir.dt.float16
F32 = mybir.dt.float32

# The last superstep splits into chunks 0-5 (768 KB), chunk 6 (128 KB),
# chunk 7 (128 KB) so only 4 concurrent N=256 matmuls + cast + 64 KB store
# remain after the last x byte. All pieces keep >=1KB per partition
# (smaller-run descriptors measured to drain pathologically slowly).


def _build_bass():
    nc = bacc.Bacc("TRN2", target_bir_lowering=False)

    # x pre-tiled on host: per-piece slab [128, (c, g, t)] so each DMA
    # moves one fully-contiguous run per partition.
    # per-superstep group width (tokens per col group)
    CGS = [NGRP * b for b in SS_BLK]
    LCG = CGS[-1]
    # full supersteps stream as two 768 KB halves (chunks 0-3 / 4-7); the
    # last superstep as chunks 0-5 (768 KB), 6 (128 KB), 7 (128 KB).
    xs_dram = []
    for s in range(SS - 1):
        xs_dram.append(
            (
                nc.dram_tensor(
                    f"xs{s}a", [128, 4 * CGS[s]], X_DT, kind="ExternalInput"
                ),
                nc.dram_tensor(
                    f"xs{s}b", [128, 4 * CGS[s]], X_DT, kind="ExternalInput"
                ),
            )
        )
    xl0 = nc.dram_tensor("xl0", [128, 6 * LCG], X_DT, kind="ExternalInput")
    xl1 = nc.dram_tensor("xl1", [128, LCG], X_DT, kind="ExternalInput")
    xl2 = nc.dram_tensor("xl2", [128, LCG], X_DT, kind="ExternalInput")
    wT = nc.dram_tensor("wT", [128, KC * M], W_DT, kind="ExternalInput")
    # out: per superstep a [NGRP*M, BLK_s] block, packed flat; row within a
    # block = 32*j + m  =  (col group j, channel m); host unpermutes.
    out_off = [0]
    for s in range(SS):
        out_off.append(out_off[-1] + SS_BLK[s])
    outT = nc.dram_tensor("outT", [NGRP * M, out_off[-1]], F16, kind="ExternalOutput")

    with tile.TileContext(nc) as tc:
        with (
            tc.tile_pool(name="w", bufs=1) as wpool,
            tc.tile_pool(name="x", bufs=1) as xpool,
            tc.tile_pool(name="out", bufs=1) as opool,
            tc.tile_pool(name="warm", bufs=1, space="PSUM") as warmpool,
            tc.tile_pool(name="psum", bufs=1, space="PSUM") as ppool,
        ):
            # w on the scalar HWDGE ring, leaving the sync ring's
            # descriptor generator free for the x stream from t=0.
            w_tile = wpool.tile([128, KC, M], W_DT)
            nc.scalar.dma_start(w_tile[:], wT.rearrange("p (c m) -> p c m", c=KC))

            # x stream on the sync ring, in consumption order.
            x_tiles = []
            for s in range(SS - 1):
                ta = xpool.tile([128, 4, NGRP, SS_BLK[s]], X_DT, tag=f"xs{s}a")
                nc.sync.dma_start(
                    ta[:],
                    xs_dram[s][0].rearrange("p (c g t) -> p c g t", c=4, g=NGRP),
                )
                tb = xpool.tile([128, 4, NGRP, SS_BLK[s]], X_DT, tag=f"xs{s}b")
                nc.sync.dma_start(
                    tb[:],
                    xs_dram[s][1].rearrange("p (c g t) -> p c g t", c=4, g=NGRP),
                )
                x_tiles.append((ta, tb))
            LB = SS_BLK[-1]
            tl0 = xpool.tile([128, 6, NGRP, LB], X_DT, tag="xl0")
            nc.sync.dma_start(
                tl0[:], xl0.rearrange("p (c g t) -> p c g t", c=6, g=NGRP)
            )
            tl1 = xpool.tile([128, NGRP, LB], X_DT, tag="xl1")
            nc.sync.dma_start(tl1[:], xl1.rearrange("p (g t) -> p g t", g=NGRP))
            tl2 = xpool.tile([128, NGRP, LB], X_DT, tag="xl2")
            nc.sync.dma_start(tl2[:], xl2.rearrange("p (g t) -> p g t", g=NGRP))

            # Warmup matmul absorbs the w-DMA wait into PE program order so
            # every real matmul needs only its x-DMA wait.
            warm = warmpool.tile([M, M], F32)
            nc.tensor.matmul(warm[:], w_tile[:, 0, :], w_tile[:, 0, :])

            # early supersteps cast into one shared SBUF tile and go out as
            # a single 192 KB store (fewer DMAs, less SDMA contention with
            # the x stream); the last superstep keeps its own 64 KB store so
            # the post-stream chain stays short.
            o_early = opool.tile([128, out_off[SS - 1]], F16, tag="oearly")
            for s in range(SS):
                blk = SS_BLK[s]
                last = s == SS - 1
                ptile = ppool.tile([128, blk], F32, tag=f"ps{s}")
                o_tile = None
                if last:
                    o_tile = opool.tile([128, blk], F16, tag="olast")
                for c in range(KC):
                    for j in range(NGRP):
                        if not last:
                            half = x_tiles[s][c // 4]
                            rhs = half[:, c % 4, j, :]
                        elif c < 6:
                            rhs = tl0[:, c, j, :]
                        elif c == 6:
                            rhs = tl1[:, j, :]
                        else:
                            rhs = tl2[:, j, :]
                        nc.tensor.matmul(
                            ptile[32 * j : 32 * (j + 1), :],
                            w_tile[:, c, :],
                            rhs,
                            start=(c == 0),
                            stop=(c == KC - 1),
                            tile_position=(0, 32 * j),
                        )

                if last:
                    nc.vector.tensor_copy(o_tile[:], ptile[:])
                    # final store on the sync ring: idle after the last x
                    # piece, while scalar may still be draining the early
                    # store's descriptors.
                    nc.sync.dma_start(
                        outT[:, out_off[s] : out_off[s + 1]], o_tile[:]
                    )
                else:
                    nc.vector.tensor_copy(
                        o_early[:, out_off[s] : out_off[s + 1]], ptile[:]
                    )
                    if s == SS - 2:
                        nc.scalar.dma_start(
                            outT[:, out_off[0] : out_off[SS - 1]], o_early[:]
                        )

    nc.compile()
    return nc


_NC_CACHE = None


def _get_nc():
    global _NC_CACHE
    if _NC_CACHE is None:
        _NC_CACHE = _build_bass()
    return _NC_CACHE


def _hadamard32() -> np.ndarray:
    h = np.array([[1.0]], dtype=np.float64)
    while h.shape[0] < M:
        h = np.block([[h, h], [h, -h]])
    return h


_NOISE_CACHE = None


def _noise() -> np.ndarray:
    # Mirror reference.py exactly (same op on the default jax backend): the
    # bits differ between backends, so the noise must be produced the same
    # way the grading reference produces it.
    global _NOISE_CACHE
    if _NOISE_CACHE is None:
        import jax

        nz = NOISE_STD * jax.random.normal(
            jax.random.key(42), (B, N, M), dtype=np.float32
        )
        _NOISE_CACHE = np.asarray(nz)
    return _NOISE_CACHE


def kernel(x: np.ndarray, W: np.ndarray, _profile_sink=None) -> np.ndarray:
    x = np.ascontiguousarray(np.asarray(x, dtype=np.float32))
    W = np.asarray(W, dtype=np.float32)

    # Fold normalized FWHT into the projection: out = x @ w_lhsT + noise
    w_eff = (_hadamard32() @ W.astype(np.float64)) / math.sqrt(M)
    w_lhsT = w_eff.T.astype(np.float16)  # [D, M]
    # pack to device SBUF layout [partition, kchunk, M]
    w_dev = np.ascontiguousarray(
        w_lhsT.reshape(KC, 128, M).transpose(1, 0, 2)
    ).reshape(128, KC * M)

    X8 = x.reshape(TOK_TOTAL, D).astype(X_NP)

    tok_off = [0]
    for t in SS_TOK:
        tok_off.append(tok_off[-1] + t)

    in_maps = []
    for i in range(N_CORES):
        base = i * TOK
        m = {"wT": w_dev}
        slabs = []
        for s in range(SS):
            sl = X8[base + tok_off[s] : base + tok_off[s + 1]]
            # [tok, d] -> [p, c, g, t] -> [128, (c g t)]
            slab = np.ascontiguousarray(
                sl.reshape(NGRP, SS_BLK[s], KC, 128)  # [g, t, c, p]
                .transpose(3, 2, 0, 1)                # [p, c, g, t]
            ).reshape(128, KC * NGRP * SS_BLK[s])
            slabs.append(slab)
        for s in range(SS - 1):
            cg = NGRP * SS_BLK[s]
            m[f"xs{s}a"] = np.ascontiguousarray(slabs[s][:, : 4 * cg])
            m[f"xs{s}b"] = np.ascontiguousarray(slabs[s][:, 4 * cg :])
        lcg = NGRP * SS_BLK[-1]
        m["xl0"] = np.ascontiguousarray(slabs[-1][:, : 6 * lcg])
        m["xl1"] = np.ascontiguousarray(slabs[-1][:, 6 * lcg : 7 * lcg])
        m["xl2"] = np.ascontiguousarray(slabs[-1][:, 7 * lcg :])
        in_maps.append(m)

    # Rare intermittent HW flakes corrupt a few hundred output elements;
    # verify the device result against the same quantized math on sampled
    # rows (cheap on host) and retry the run if corruption is detected.
    chk_rows = np.arange(0, TOK_TOTAL, 61)
    chk_ref = X8[chk_rows].astype(np.float32) @ w_lhsT.astype(np.float32)

    out_off = [0]
    for s in range(SS):
        out_off.append(out_off[-1] + SS_BLK[s])

    out = None
    for _attempt in range(3):
        res = run_bass_kernel_spmd(
            _get_nc(),
            in_maps,
            core_ids=list(range(N_CORES)),
            trace=_profile_sink is not None,
        )
        if _profile_sink is not None:
            _profile_sink.append(res)

        outs = []
        for r in res.results:
            o = r["outT"].astype(np.float32)      # [NGRP*M, sum(BLK)]
            parts = []
            for s in range(SS):
                blk = o[:, out_off[s] : out_off[s + 1]]        # [(j m), t]
                parts.append(
                    blk.reshape(NGRP, M, SS_BLK[s])
                    .transpose(0, 2, 1)
                    .reshape(SS_TOK[s], M)
                )
            outs.append(np.concatenate(parts, axis=0))
        out = np.concatenate(outs, axis=0)
        if np.abs(out[chk_rows] - chk_ref).max() < 0.05:
            break

    out = out + _noise().reshape(TOK_TOTAL, M)
    return np.ascontiguousarray(out.reshape(B, N, M).astype(np.float32))


if __name__ == "__main__":
    xs = np.random.randn(B, N, D).astype(np.float32)
    Ws = (np.random.randn(M, D) / math.sqrt(D)).astype(np.float32)
    o = kernel(xs, Ws)
    print(o.shape, o.dtype)


# revision 65
# speedup vs baseline: 1.2399x; 1.2399x over previous
"""Trainium2 Bass kernel for ContinuousREWAEncoder:
    out = FWHT(x @ W^T)/sqrt(32) + 0.01*normal(key=42)

Math folding: FWHT is linear => out = x @ (H @ W / sqrt(32))^T + noise.
The noise uses a fixed PRNG key, so it is a deterministic constant computed
on host (with the same jax op/backend as the reference) and added in the
host epilogue (with the layout unpermute), keeping it off the HBM stream.

Sharding: pure data parallel over tokens (B*N = 32768 -> 4096/core on 8
cores). W_eff is replicated.

The kernel is HBM-bound, so x streams as fp8e3 (e3m4: 4 mantissa bits) —
half the bytes of fp16 — while W stays fp16 (mixed-dtype matmul). Measured
absmax rel err vs the fp32 reference ~1.1e-2 (gate 2e-2). Output moves as
fp16.

Device schedule per core (TOK=4096, supersteps of 1536/1536/1024 tokens,
each = 4 PE column groups):
  - all x rides the sync HWDGE ring as 7 DMAs in consumption order
    (768 KB halves of the full supersteps; the last superstep as
    chunks 0-5 [768 KB], chunk 6 [128 KB], chunk 7 [128 KB]) so only a
    small piece arrives last. Pieces stay >=1 KB per partition —
    smaller-run descriptors measured intermittent multi-us drain crawls.
  - w and the early out store ride the scalar HWDGE ring; the final
    64 KB store rides sync (idle once the last x piece drains, while
    scalar may still hold the early store's descriptors). Splitting the
    x stream itself across both rings measured WORSE: SDMA service
    between queues is unfair and the whole-DMA completion semaphore
    turned a trickling scalar-side piece into a 4us PE stall.
  - per superstep: 8 k-chunks x 4 col groups of matmuls (N = tokens/4),
    the 4 groups running concurrently in the PE column groups
    (tile_position), accumulating into that superstep's own PSUM bank;
    then a [128, N] DVE cast. Separate banks let each superstep's
    cast pipeline under the continuing x stream (PE-write/DVE-read on
    one bank serialize, so sub-bank splits can't).
  - three supersteps keep the PE program at ~200 instructions — under
    one 16 KiB IRAM block. Four or more supersteps overflowed it and a
    mid-kernel instruction fetch stalled the PE 1-3us at the tail
    (run-to-run variance of +-2us; now +-0.3us).
  - supersteps 0-1 cast into one shared SBUF tile stored as a single
    192 KB DMA; the last superstep stores separately (64 KB) so after
    the final x byte only 4 concurrent N=256 matmuls, one [128,256]
    cast and the 64 KB store remain (~2.8us tail).
  - measured anatomy per run (~25.5us total): ~6.6us fixed runtime/NEFF
    entry (engine init + program load + barriers), ~1.45us issue-to-
    first-byte, ~11.5us x stream at 360-430 GB/s (HBM-bound), ~2.8us
    tail chain, ~2.8us fixed exit (drain + semaphore teardown).
"""

import math
import sys
import types

import numpy as np
import ml_dtypes

import concourse.tile as tile
from concourse import bacc, mybir
from concourse.bass_utils import run_bass_kernel_spmd


def _ensure_axon_hooks():
    """run_bass_kernel_spmd's trace path (taken whenever env BASS_TRACE is
    set, even without an explicit trace=True) imports antenv.axon_hooks,
    which this image's antenv stub lacks. Provide it so the kernel works
    regardless of the caller's environment; register the real NTFF hook if
    the boot helper is available (hook=None degrades to a no-trace run)."""
    try:
        import antenv

        if not hasattr(antenv, "axon_hooks"):
            mod = types.ModuleType("antenv.axon_hooks")
            mod._hook = None
            mod.set_axon_ntff_profile_hook = lambda h: setattr(mod, "_hook", h)
            mod.get_axon_ntff_profile_hook = lambda: mod._hook
            sys.modules["antenv.axon_hooks"] = mod
            antenv.axon_hooks = mod
            # hook stays None: run_bass_kernel_spmd then degrades to an
            # untraced run instead of crashing on the missing module. A
            # harness that wants NTFF traces registers its own hook here.
    except Exception:
        pass


_ensure_axon_hooks()

B, N, D, M = 4, 8192, 1024, 32
NOISE_STD = 0.01
N_CORES = 8
TOK_TOTAL = B * N              # 32768
TOK = TOK_TOTAL // N_CORES     # 4096 tokens per core
NGRP = 4                       # col groups per superstep (PE col tiling)
KC = D // 128                  # 8 contraction chunks
# Three supersteps keep the PE program under 256 instructions (one 16 KiB
# IRAM block): 96 matmuls + 96 ldweights. Four supersteps measured an
# intermittent mid-kernel IRAM refill (16 KB runtime DMA late in the
# stream) that stalls the PE 1-3us right at the tail.
SS_TOK = [1536, 1536, 1024]             # tokens per superstep
SS_BLK = [t // NGRP for t in SS_TOK]    # 384, 384, 256
SS = len(SS_TOK)
assert sum(SS_TOK) == TOK

X_DT = mybir.dt.float8e3       # e3m4: 1 byte, 4 mantissa bits
X_NP = ml_dtypes.float8_e3m4
W_DT = mybir.dt.float16
F16 = mybir.dt.float16
F32 = mybir.dt.float32

# The last superstep splits into chunks 0-5 (768 KB), chunk 6 (128 KB),
# chunk 7 (128 KB) so only 4 concurrent N=256 matmuls + cast + 64 KB store
# remain after the last x byte. All pieces keep >=1KB per partition
# (smaller-run descriptors measured to drain pathologically slowly).


def _build_bass():
    nc = bacc.Bacc(
        "TRN2",
        target_bir_lowering=False,
        enable_partition_id=False,
        monotonic_sem_count=0,
    )

    # x pre-tiled on host: per-piece slab [128, (c, g, t)] so each DMA
    # moves one fully-contiguous run per partition.
    # per-superstep group width (tokens per col group)
    CGS = [NGRP * b for b in SS_BLK]
    LCG = CGS[-1]
    # full supersteps stream as two 768 KB halves (chunks 0-3 / 4-7); the
    # last superstep as chunks 0-5 (768 KB), 6 (128 KB), 7 (128 KB).
    xs_dram = []
    for s in range(SS - 1):
        xs_dram.append(
            (
                nc.dram_tensor(
                    f"xs{s}a", [128, 4 * CGS[s]], X_DT, kind="ExternalInput"
                ),
                nc.dram_tensor(
                    f"xs{s}b", [128, 4 * CGS[s]], X_DT, kind="ExternalInput"
                ),
            )
        )
    xl0 = nc.dram_tensor("xl0", [128, 6 * LCG], X_DT, kind="ExternalInput")
    xl1 = nc.dram_tensor("xl1", [128, LCG], X_DT, kind="ExternalInput")
    xl2 = nc.dram_tensor("xl2", [128, LCG], X_DT, kind="ExternalInput")
    wT = nc.dram_tensor("wT", [128, KC * M], W_DT, kind="ExternalInput")
    # out: per superstep a [NGRP*M, BLK_s] block, packed flat; row within a
    # block = 32*j + m  =  (col group j, channel m); host unpermutes.
    out_off = [0]
    for s in range(SS):
        out_off.append(out_off[-1] + SS_BLK[s])
    outT = nc.dram_tensor("outT", [NGRP * M, out_off[-1]], F16, kind="ExternalOutput")

    with tile.TileContext(nc) as tc:
        with (
            tc.tile_pool(name="w", bufs=1) as wpool,
            tc.tile_pool(name="x", bufs=1) as xpool,
            tc.tile_pool(name="out", bufs=1) as opool,
            tc.tile_pool(name="warm", bufs=1, space="PSUM") as warmpool,
            tc.tile_pool(name="psum", bufs=1, space="PSUM") as ppool,
        ):
            # w on the scalar HWDGE ring, leaving the sync ring's
            # descriptor generator free for the x stream from t=0.
            w_tile = wpool.tile([128, KC, M], W_DT)
            nc.scalar.dma_start(w_tile[:], wT.rearrange("p (c m) -> p c m", c=KC))

            # x stream on the sync ring, in consumption order.
            x_tiles = []
            for s in range(SS - 1):
                ta = xpool.tile([128, 4, NGRP, SS_BLK[s]], X_DT, tag=f"xs{s}a")
                nc.sync.dma_start(
                    ta[:],
                    xs_dram[s][0].rearrange("p (c g t) -> p c g t", c=4, g=NGRP),
                )
                tb = xpool.tile([128, 4, NGRP, SS_BLK[s]], X_DT, tag=f"xs{s}b")
                nc.sync.dma_start(
                    tb[:],
                    xs_dram[s][1].rearrange("p (c g t) -> p c g t", c=4, g=NGRP),
                )
                x_tiles.append((ta, tb))
            LB = SS_BLK[-1]
            tl0 = xpool.tile([128, 6, NGRP, LB], X_DT, tag="xl0")
            nc.sync.dma_start(
                tl0[:], xl0.rearrange("p (c g t) -> p c g t", c=6, g=NGRP)
            )
            tl1 = xpool.tile([128, NGRP, LB], X_DT, tag="xl1")
            nc.sync.dma_start(tl1[:], xl1.rearrange("p (g t) -> p g t", g=NGRP))
            tl2 = xpool.tile([128, NGRP, LB], X_DT, tag="xl2")
            nc.sync.dma_start(tl2[:], xl2.rearrange("p (g t) -> p g t", g=NGRP))

            # Warmup matmul absorbs the w-DMA wait into PE program order so
            # every real matmul needs only its x-DMA wait.
            warm = warmpool.tile([M, M], F32)
            nc.tensor.matmul(warm[:], w_tile[:, 0, :], w_tile[:, 0, :])

            # early supersteps cast into one shared SBUF tile and go out as
            # a single 192 KB store (fewer DMAs, less SDMA contention with
            # the x stream); the last superstep keeps its own 64 KB store so
            # the post-stream chain stays short.
            o_early = opool.tile([128, out_off[SS - 1]], F16, tag="oearly")
            for s in range(SS):
                blk = SS_BLK[s]
                last = s == SS - 1
                ptile = ppool.tile([128, blk], F32, tag=f"ps{s}")
                o_tile = None
                if last:
                    o_tile = opool.tile([128, blk], F16, tag="olast")
                for c in range(KC):
                    for j in range(NGRP):
                        if not last:
                            half = x_tiles[s][c // 4]
                            rhs = half[:, c % 4, j, :]
                        elif c < 6:
                            rhs = tl0[:, c, j, :]
                        elif c == 6:
                            rhs = tl1[:, j, :]
                        else:
                            rhs = tl2[:, j, :]
                        nc.tensor.matmul(
                            ptile[32 * j : 32 * (j + 1), :],
                            w_tile[:, c, :],
                            rhs,
                            start=(c == 0),
                            stop=(c == KC - 1),
                            tile_position=(0, 32 * j),
                        )

                if last:
                    nc.vector.tensor_copy(o_tile[:], ptile[:])
                    # final store on the sync ring: idle after the last x
                    # piece, while scalar may still be draining the early
                    # store's descriptors.
                    nc.sync.dma_start(
                        outT[:, out_off[s] : out_off[s + 1]], o_tile[:]
                    )
                else:
                    nc.vector.tensor_copy(
                        o_early[:, out_off[s] : out_off[s + 1]], ptile[:]
                    )
                    if s == SS - 2:
                        nc.scalar.dma_start(
                            outT[:, out_off[0] : out_off[SS - 1]], o_early[:]
                        )

    nc.compile()
    return nc


_NC_CACHE = None


def _get_nc():
    global _NC_CACHE
    if _NC_CACHE is None:
        _NC_CACHE = _build_bass()
    return _NC_CACHE


def _hadamard32() -> np.ndarray:
    h = np.array([[1.0]], dtype=np.float64)
    while h.shape[0] < M:
        h = np.block([[h, h], [h, -h]])
    return h


_NOISE_CACHE = None


def _noise() -> np.ndarray:
    # Mirror reference.py exactly (same op on the default jax backend): the
    # bits differ between backends, so the noise must be produced the same
    # way the grading reference produces it.
    global _NOISE_CACHE
    if _NOISE_CACHE is None:
        import jax

        nz = NOISE_STD * jax.random.normal(
            jax.random.key(42), (B, N, M), dtype=np.float32
        )
        _NOISE_CACHE = np.asarray(nz)
    return _NOISE_CACHE


def kernel(x: np.ndarray, W: np.ndarray, _profile_sink=None) -> np.ndarray:
    x = np.ascontiguousarray(np.asarray(x, dtype=np.float32))
    W = np.asarray(W, dtype=np.float32)

    # Fold normalized FWHT into the projection: out = x @ w_lhsT + noise
    w_eff = (_hadamard32() @ W.astype(np.float64)) / math.sqrt(M)
    w_lhsT = w_eff.T.astype(np.float16)  # [D, M]
    # pack to device SBUF layout [partition, kchunk, M]
    w_dev = np.ascontiguousarray(
        w_lhsT.reshape(KC, 128, M).transpose(1, 0, 2)
    ).reshape(128, KC * M)

    X8 = x.reshape(TOK_TOTAL, D).astype(X_NP)

    tok_off = [0]
    for t in SS_TOK:
        tok_off.append(tok_off[-1] + t)

    in_maps = []
    for i in range(N_CORES):
        base = i * TOK
        m = {"wT": w_dev}
        slabs = []
        for s in range(SS):
            sl = X8[base + tok_off[s] : base + tok_off[s + 1]]
            # [tok, d] -> [p, c, g, t] -> [128, (c g t)]
            slab = np.ascontiguousarray(
                sl.reshape(NGRP, SS_BLK[s], KC, 128)  # [g, t, c, p]
                .transpose(3, 2, 0, 1)                # [p, c, g, t]
            ).reshape(128, KC * NGRP * SS_BLK[s])
            slabs.append(slab)
        for s in range(SS - 1):
            cg = NGRP * SS_BLK[s]
            m[f"xs{s}a"] = np.ascontiguousarray(slabs[s][:, : 4 * cg])
            m[f"xs{s}b"] = np.ascontiguousarray(slabs[s][:, 4 * cg :])
        lcg = NGRP * SS_BLK[-1]
        m["xl0"] = np.ascontiguousarray(slabs[-1][:, : 6 * lcg])
        m["xl1"] = np.ascontiguousarray(slabs[-1][:, 6 * lcg : 7 * lcg])
        m["xl2"] = np.ascontiguousarray(slabs[-1][:, 7 * lcg :])
        in_maps.append(m)

    # Rare intermittent HW flakes corrupt a few hundred output elements;
    # verify the device result against the same quantized math on sampled
    # rows (cheap on host) and retry the run if corruption is detected.
    chk_rows = np.arange(0, TOK_TOTAL, 61)
    chk_ref = X8[chk_rows].astype(np.float32) @ w_lhsT.astype(np.float32)

    out_off = [0]
    for s in range(SS):
        out_off.append(out_off[-1] + SS_BLK[s])

    out = None
    for _attempt in range(3):
        res = run_bass_kernel_spmd(
            _get_nc(),
            in_maps,
            core_ids=list(range(N_CORES)),
            trace=_profile_sink is not None,
        )
        if _profile_sink is not None:
            _profile_sink.append(res)

        outs = []
        for r in res.results:
            o = r["outT"].astype(np.float32)      # [NGRP*M, sum(BLK)]
            parts = []
            for s in range(SS):
                blk = o[:, out_off[s] : out_off[s + 1]]        # [(j m), t]
                parts.append(
                    blk.reshape(NGRP, M, SS_BLK[s])
                    .transpose(0, 2, 1)
                    .reshape(SS_TOK[s], M)
                )
            outs.append(np.concatenate(parts, axis=0))
        out = np.concatenate(outs, axis=0)
        if np.abs(out[chk_rows] - chk_ref).max() < 0.05:
            break

    out = out + _noise().reshape(TOK_TOTAL, M)
    return np.ascontiguousarray(out.reshape(B, N, M).astype(np.float32))


if __name__ == "__main__":
    xs = np.random.randn(B, N, D).astype(np.float32)
    Ws = (np.random.randn(M, D) / math.sqrt(D)).astype(np.float32)
    o = kernel(xs, Ws)
    print(o.shape, o.dtype)
